# revision 30
# baseline (speedup 1.0000x reference)
"""Trainium2 Bass kernel for nn_CLIP_Embedding_35613868818658.

CNN stem (3x conv1d+GroupNorm+ReLU, 768->128->256->512) -> LayerNorm ->
bidirectional Mamba (selective scan, d_inner=1024, d_state=16, L=1024) ->
out_proj + residual.  Output (2, 512, 1024) f32.

Sharding: 2 batch-groups x 4-way d_inner split (DSH=256 rows per core).
Cores 0-3 handle b=0, cores 4-7 handle b=1; core g within a group owns
d_inner rows [256g, 256(g+1)).  Two in-group collectives, both bf16:
AllReduce of x_dbl (B/C/dt projections, contracted over sharded d_inner)
and a ReduceScatter of out_proj partials (+res/4), so each core emits its
own 128-row quarter of the output and the host concatenates.

The selective scan runs as 16 (one per state index s) tensor_tensor_scan
instructions per d-tile over a [128, 2048] layout that concatenates the
forward and (time-reversed) backward directions along the free axis;
m[, t=0|1024] = -30 makes a = exp((s+1)m) ~ 0, resetting the recurrence at
segment starts.  The s-contraction y = sum_s C_s*h_s runs on the PE as
PSUM-accumulated identity matmuls (plus diag(D) matmuls for the D-term);
scans are split DVE/Pool to balance engine load.
"""

import numpy as np
import ml_dtypes

import concourse.bass as bass
import concourse.mybir as mybir
import concourse.tile as tile
from contextlib import ExitStack

BF16 = ml_dtypes.bfloat16
F32 = mybir.dt.float32
BF = mybir.dt.bfloat16

B, CIN, L = 2, 768, 1024
DM, DI, DS, DTR, DC = 512, 1024, 16, 32, 4
NCORES, NGRP = 8, 4
DSH = DI // NGRP          # 256 d_inner rows per core
NDT = DSH // 128          # 2 d-tiles of 128 partitions
T2 = 2 * L                # fwd|rev concatenated time axis
EPS = 1e-5
# walrus only lowers tensor_tensor_scan on DVE; plain TensorTensor mults are
# Pool-legal, so engine balance comes from sending ~45% of the b/g multiplies
# to Pool (Pool mult is ~3.7x slower than DVE's 2x-mode mult).
POOL_BS = frozenset((1, 3, 5, 7, 9, 11, 13))
POOL_GS = frozenset((0, 2, 4, 6, 8, 10, 12))

# timing-only probes (wrong numerics): 'nobcast' hoists the B/C broadcast
# out of the s-loop; 'noscan' replaces tensor_tensor_scan with a plain mult
PROBE = None

AluOp = mybir.AluOpType
ActFn = mybir.ActivationFunctionType


def _ap_bcast_dram(handle, offset, dims):
    """Raw AP on a DRAM tensor: dims is a list of [step, count]."""
    return bass.AP(tensor=handle, offset=offset, ap=[list(d) for d in dims])


def split_excess_waits(nc, max_waits=1):
    """Walrus rejects instructions carrying more sync waits than the ISA
    encoding has slots for (1 on this toolchain).  Move excess waits onto
    preceding same-engine NoOps."""
    for bb in nc.main_func.blocks:
        insts = bb.instructions
        out, changed = [], False
        for ins in insts:
            si = ins.sync_info
            if si is not None and si.on_wait is not None and len(si.on_wait) > max_waits:
                waits = list(si.on_wait)
                keep, rest = waits[:max_waits], waits[max_waits:]
                idx = 0
                while rest:
                    chunk, rest = rest[:max_waits], rest[max_waits:]
                    nop = mybir.InstNoOp(
                        name=f"{ins.name}-wsplit{idx}",
                        engine=ins.engine,
                        sync_info=mybir.SyncInfo(on_wait=chunk, on_update=[]),
                        bass_nofuse=True,
                    )
                    out.append(nop)
                    idx += 1
                ins.sync_info = mybir.SyncInfo(
                    on_wait=keep, on_update=list(si.on_update or [])
                )
                changed = True
            out.append(ins)
        if changed:
            bb.instructions = out
    return nc


def build_program(a_vals, split_waits=True, debug_dumps=False, reps=1, upto='full'):
    """Build the SPMD Bass program.  a_vals: 16 negative floats, A[s] = -(s+1)
    (verified d-independent and equal for both directions on the host)."""
    nc = bass.Bass("TRN2", target_bir_lowering=False, debug=False,
                   num_devices=NCORES)

    dt_in = lambda n, s, d=BF: nc.dram_tensor(n, list(s), d, kind="ExternalInput")

    x_in = dt_in("x", (CIN, L + 2))                      # host-padded, bf16
    w1T = dt_in("w1T", (3, 6, 128, 128))
    w2T = dt_in("w2T", (3, 1, 128, 256))
    w3T = dt_in("w3T", (3, 2, 128, 512))
    cb1 = dt_in("cb1", (128, 1), F32)
    cb2 = dt_in("cb2", (256, 1), F32)
    cb3 = dt_in("cb3", (512, 1), F32)
    gng1 = dt_in("gng1", (128, 1), F32)
    gnb1 = dt_in("gnb1", (128, 1), F32)
    gng2 = dt_in("gng2", (256, 1), F32)
    gnb2 = dt_in("gnb2", (256, 1), F32)
    gng3 = dt_in("gng3", (512, 1), F32)
    gnb3 = dt_in("gnb3", (512, 1), F32)
    onehot = dt_in("onehot", (3, 128, 32))
    onehotE = dt_in("onehotE", (3, 32, 128))
    ones_col = dt_in("ones_col", (128, 1))
    inprojT = dt_in("inprojT", (4, 128, 512))
    augT = dt_in("augT", (2, 512))
    xpT = dt_in("xpT", (2, 2, 128, 64))                 # [dir][ktile]
    dtT = dt_in("dtT", (2, 32, 256))                    # [dir]
    ndtb = dt_in("ndtb", (2, 256, 1), F32)              # -dt_b
    cvwD = dt_in("cvwD", (2, 2, 4, 128, 128))          # [dir][dtile][k] diag
    cvb = dt_in("cvb", (2, 256, 1), F32)
    outT = dt_in("outT", (2, 128, 512))                 # [dtile]
    identw = dt_in("identw", (128, 128))
    dDiag = dt_in("dDiag", (2, 2, 128, 128))            # [dir][dtile] diag(D)

    out_ext = nc.dram_tensor("out", [128, L], BF, kind="ExternalOutput")

    with tile.TileContext(nc) as tc, ExitStack() as ctx:
        P = 128
        consts = ctx.enter_context(tc.tile_pool(name="consts", bufs=1))
        psum = ctx.enter_context(tc.tile_pool(name="psum", bufs=2, space="PSUM"))
        mid = ctx.enter_context(tc.tile_pool(name="mid", bufs=1))
        dram = ctx.enter_context(tc.tile_pool(name="dram", bufs=1, space="DRAM"))
        sync, vec, pool, act, pe = nc.sync, nc.vector, nc.gpsimd, nc.scalar, nc.tensor

        # ---------------- consts to SBUF ----------------
        def load(poolh, shape, src, dtype=BF, name=None):
            t = poolh.tile(list(shape), dtype, tag=name)
            sync.dma_start(t[:], src)
            return t

        w1 = [[load(consts, (P, 128), w1T[k, ct], name=f"w1_{k}_{ct}")
               for ct in range(6)] for k in range(3)]
        w2 = [[load(consts, (P, 256), w2T[k, ct], name=f"w2_{k}_{ct}")
               for ct in range(1)] for k in range(3)]
        w3 = [[load(consts, (P, 512), w3T[k, ct], name=f"w3_{k}_{ct}")
               for ct in range(2)] for k in range(3)]
        def load_cols(dramt, co, name, width=1):
            return [load(consts, (128, width), dramt[mt * 128:(mt + 1) * 128, :],
                         F32, f"{name}{mt}") for mt in range(co // 128)]

        cbs = [load_cols(cb1, 128, "cb1"), load_cols(cb2, 256, "cb2"),
               load_cols(cb3, 512, "cb3")]
        gngs = [load_cols(gng1, 128, "gng1"), load_cols(gng2, 256, "gng2"),
                load_cols(gng3, 512, "gng3")]
        gnbs = [load_cols(gnb1, 128, "gnb1"), load_cols(gnb2, 256, "gnb2"),
                load_cols(gnb3, 512, "gnb3")]
        oneh = [load(consts, (P, 32), onehot[i], name=f"onehot{i}")
                for i in range(3)]
        onehE = [load(consts, (32, 128), onehotE[i], name=f"onehotE{i}")
                 for i in range(3)]
        ones1 = load(consts, (P, 1), ones_col[:], name="ones1")
        ipT = [load(consts, (P, 512), inprojT[kt], name=f"ipT{kt}") for kt in range(4)]
        augTs = load(consts, (2, 512), augT[:], name="augT")
        xpTs = [[load(consts, (P, 64), xpT[d, kt], name=f"xpT{d}{kt}")
                 for kt in range(2)] for d in range(2)]
        dtTs = [load(consts, (32, 256), dtT[d], name=f"dtT{d}") for d in range(2)]
        ndtbs = [[load(consts, (128, 1), ndtb[d, dt * 128:(dt + 1) * 128, :], F32,
                       f"ndtb{d}{dt}") for dt in range(2)] for d in range(2)]
        cvwDs = [[[load(consts, (P, 128), cvwD[d, dt, k], name=f"cvwD{d}{dt}{k}")
                   for k in range(4)] for dt in range(2)] for d in range(2)]
        cvbs = [[load(consts, (128, 1), cvb[d, dt * 128:(dt + 1) * 128, :], F32,
                      f"cvb{d}{dt}") for dt in range(2)] for d in range(2)]
        outTs = [load(consts, (P, 512), outT[dt], name=f"outT{dt}") for dt in range(2)]
        ident = load(consts, (P, 128), identw[:], name="ident")
        dDs = [[load(consts, (P, 128), dDiag[d, dt], name=f"dD{d}{dt}")
                for dt in range(2)] for d in range(2)]

        epsc = consts.tile([128, 1], F32, tag="epsc")
        vec.memset(epsc[:], EPS)

        # res spill: written by A(rep), read back by B2(rep) two rounds later
        res_dram = [dram.tile([DM, L], BF, tag=f"res_dram{r}", name="g")
                    for r in range(3)]
        # DRAM scratch, double-buffered so rep i+1 overlaps rep i
        scr = []
        for rp in range(2):
            scr.append(dict(
                ln_scr=dram.tile([1, L], BF, tag=f"ln_scr{rp}", name="g"),
                xdbl_loc=dram.tile([2, 64, L], BF, tag=f"xdbl_loc{rp}", name="g"),
                xdbl_red=dram.tile([2, 64, L], BF, tag=f"xdbl_red{rp}", name="g"),
                out_loc=dram.tile([DM, L], BF, tag=f"out_loc{rp}", name="g"),
                out_rs=dram.tile([128, L], BF, tag=f"out_rs{rp}", name="g"),
            ))

        def dbg_out(src_ap):
            t = mid.tile([P, L], BF, tag="dbg_cast", name="dbg_cast")
            act.activation(t[:], src_ap if isinstance(src_ap, bass.AP) else src_ap[:],
                           ActFn.Copy)
            sync.dma_start(out_ext[:], t[:])

        scanp = ctx.enter_context(tc.tile_pool(name="scanp", bufs=2))
        onep = ctx.enter_context(tc.tile_pool(name="onep", bufs=1))

        def phase_A(rep, out_st):
            """Stem + LayerNorm + in_proj (generator: yields between conv
            layers so the driver can interleave emission with the scan of an
            earlier rep).  Writes res_dram (rep%3), xpad/z (rep%2)."""
            rp2, rp3 = rep % 2, rep % 3
            ln_scr = scr[rep % 2]["ln_scr"]
            fctx = ExitStack()
            stem = fctx.enter_context(tc.tile_pool(name=f"stem{rep}", bufs=1))
            stemtmp = fctx.enter_context(tc.tile_pool(name=f"stemtmp{rep}", bufs=2))
            statp = fctx.enter_context(tc.tile_pool(name=f"statp{rep}", bufs=2))
            rows = fctx.enter_context(tc.tile_pool(name=f"rows{rep}", bufs=1))
            x_t = [load(stem, (P, L + 2), x_in[ct * P:(ct + 1) * P, :],
                        name=f"x{ct}") for ct in range(6)]
            # ---------------- CNN stem ----------------
            def conv_gn_relu(layer, in_tiles, ws, cb, gng, gnb, co, to_mid):
                # generator: yields after each 128-channel tile
                """in_tiles: list of padded (128, L+2) bf16; returns list of
                normalized+relu'd output tiles.  to_mid: final layer (res)."""
                n_ct = len(in_tiles)
                n_co = co // 128
                cg = co // 32            # channels per group
                ngt = 128 // cg          # groups per 128-channel tile
                group_elems = float(cg) * L
                outs = []
                for mt in range(n_co):
                    h_raw = stemtmp.tile([P, L], F32, tag="h_raw", bufs=1)
                    stat4 = statp.tile([P, 4], F32, tag="stat4")
                    sq = stemtmp.tile([P, 512], BF, tag="sq")
                    for n in range(2):
                        ps = psum.tile([P, 512], F32, tag="ps_main", name="ps")
                        nmm = n_ct * 3
                        i = 0
                        for ct in range(n_ct):
                            for k in range(3):
                                pe.matmul(
                                    ps[:],
                                    ws[k][ct][:, mt * 128:(mt + 1) * 128],
                                    in_tiles[ct][:, n * 512 + k: n * 512 + k + 512],
                                    start=(i == 0), stop=(i == nmm - 1),
                                )
                                i += 1
                        act.activation(h_raw[:, n * 512:(n + 1) * 512], ps[:],
                                       ActFn.Identity, bias=cb[mt][:],
                                       accum_out=stat4[:, n:n + 1])
                        act.activation(sq[:], h_raw[:, n * 512:(n + 1) * 512],
                                       ActFn.Square, accum_out=stat4[:, 2 + n:3 + n])
                    # group stats: per-partition sums -> per-group via one-hot matmul
                    stat4b = statp.tile([P, 4], BF, tag="stat4b")
                    vec.tensor_copy(stat4b[:], stat4[:])
                    gps = psum.tile([32, 4], F32, tag="ps_row", name="gps", bufs=2)
                    pe.matmul(gps[:], oneh[layer - 1][:], stat4b[:])
                    gsb = statp.tile([32, 4], F32, tag="gsb")
                    act.activation(gsb[:], gps[:], ActFn.Copy)
                    sx = statp.tile([32, 1], F32, tag="sx")
                    sq_g = statp.tile([32, 1], F32, tag="sq_g")
                    vec.tensor_add(sx[:], gsb[:, 0:1], gsb[:, 1:2])
                    vec.tensor_add(sq_g[:], gsb[:, 2:3], gsb[:, 3:4])
                    mean = statp.tile([32, 1], F32, tag="mean")
                    act.activation(mean[:], sx[:], ActFn.Copy, scale=1.0 / group_elems)
                    msq = statp.tile([32, 1], F32, tag="msq")
                    act.activation(msq[:], sx[:], ActFn.Square, scale=1.0 / group_elems)
                    var = statp.tile([32, 1], F32, tag="var")
                    vec.scalar_tensor_tensor(var[:], sq_g[:], 1.0 / group_elems, msq[:],
                                             AluOp.mult, AluOp.subtract)
                    sig_g = statp.tile([32, 1], F32, tag="sig_g")
                    act.activation(sig_g[:], var[:], ActFn.Sqrt, bias=epsc[:32, :])
                    rstd = statp.tile([32, 1], F32, tag="rstd")
                    vec.reciprocal(rstd[:], sig_g[:])
                    # pack [rstd, mean]; expand groups 32 -> channels 128 via
                    # a one-hot matmul (no DRAM round-trip)
                    stat2 = statp.tile([32, 2], BF, tag="stat2")
                    vec.tensor_copy(stat2[:, 0:1], rstd[:])
                    vec.tensor_copy(stat2[:, 1:2], mean[:])
                    ch2p = psum.tile([P, 2], F32, tag="ps_row", name="ch2p", bufs=2)
                    pe.matmul(ch2p[:], onehE[layer - 1][:], stat2[:])
                    scale_c = statp.tile([P, 1], F32, tag="scale_c")
                    vec.tensor_mul(scale_c[:], ch2p[:, 0:1], gng[mt][:])
                    nmean_s = statp.tile([P, 1], F32, tag="nmean_s")
                    vec.tensor_mul(nmean_s[:], ch2p[:, 1:2], scale_c[:])
                    bias_c = statp.tile([P, 1], F32, tag="bias_c")
                    vec.tensor_sub(bias_c[:], gnb[mt][:], nmean_s[:])
                    if to_mid:
                        h_out = stem.tile([P, L], BF, tag=f"res{mt}")
                        act.activation(h_out[:], h_raw[:], ActFn.Relu,
                                       scale=scale_c[:], bias=bias_c[:])
                        sync.dma_start(res_dram[rp3][mt * 128:(mt + 1) * 128, :],
                                       h_out[:])
                    else:
                        h_out = stem.tile([P, L + 2], BF, tag=f"h{layer}_{mt}")
                        vec.memset(h_out[:, 0:1], 0.0)
                        vec.memset(h_out[:, L + 1:L + 2], 0.0)
                        act.activation(h_out[:, 1:L + 1], h_raw[:], ActFn.Relu,
                                       scale=scale_c[:], bias=bias_c[:])
                    outs.append(h_out)
                    yield
                return outs

            h1 = yield from conv_gn_relu(1, x_t, w1, cbs[0], gngs[0], gnbs[0], 128, False)
            h2 = yield from conv_gn_relu(2, h1, w2, cbs[1], gngs[1], gnbs[1], 256, False)
            res = yield from conv_gn_relu(3, h2, w3, cbs[2], gngs[2], gnbs[2], 512, True)
            out_st["rp3"] = rp3
            yield

            if upto == 'stem':
                dbg_out(res[0])
                fctx.close()
                return
            # ---------------- LayerNorm stats (over channels, via matmuls) -------
            hsq = []
            for mt in range(4):
                t = stemtmp.tile([P, L], BF, tag="hsq")
                act.activation(t[:], res[mt][:], ActFn.Square)
                hsq.append(t)
            musum = rows.tile([1, L], BF, tag="musum")
            sqsum = rows.tile([1, L], BF, tag="sqsum")
            for n in range(2):
                mu_ps = psum.tile([1, 512], F32, tag="ps_row", name="mu_ps", bufs=2)
                for kt in range(4):
                    pe.matmul(mu_ps[:], ones1[:],
                              res[kt][:, n * 512:(n + 1) * 512],
                              start=(kt == 0), stop=(kt == 3))
                act.activation(musum[:, n * 512:(n + 1) * 512], mu_ps[:], ActFn.Copy)
                sq_ps = psum.tile([1, 512], F32, tag="ps_row", name="sq_ps", bufs=2)
                for kt in range(4):
                    pe.matmul(sq_ps[:], ones1[:],
                              hsq[kt][:, n * 512:(n + 1) * 512],
                              start=(kt == 0), stop=(kt == 3))
                act.activation(sqsum[:, n * 512:(n + 1) * 512], sq_ps[:], ActFn.Copy)
            msql = rows.tile([1, L], BF, tag="msql")
            act.activation(msql[:], musum[:], ActFn.Square, scale=1.0 / DM)
            varl = rows.tile([1, L], BF, tag="varl")
            vec.scalar_tensor_tensor(varl[:], sqsum[:], 1.0 / DM, msql[:],
                                     AluOp.mult, AluOp.subtract)
            sigma = rows.tile([1, L], BF, tag="sigma")
            act.activation(sigma[:], varl[:], ActFn.Sqrt, bias=epsc[:1, :])
            recip = rows.tile([1, L], BF, tag="msql", name="recip")
            with nc.allow_low_precision(reason="LN 1/sigma in bf16; |err|~4e-3 ok"):
                vec.reciprocal(recip[:], sigma[:])
            nmu_b = rows.tile([1, L], BF, tag="varl", name="nmu_b")
            act.activation(nmu_b[:], musum[:], ActFn.Identity, scale=-1.0 / DM)
            aug = rows.tile([2, L], BF, tag="aug")
            sync.dma_start(aug[0:1, :], nmu_b[:])
            sync.dma_start(aug[1:2, :], sigma[:])
            sync.dma_start(ln_scr[:], recip[:])
            rbc = rows.tile([P, L], BF, tag="rbc")
            sync.dma_start(
                rbc[:],
                _ap_bcast_dram(ln_scr[:].tensor, ln_scr[:].offset, [[0, P], [1, L]]),
            )

            # ---------------- in_proj (LN folded in) ----------------
            # xpad[dt]: (128, L+6) bf16, 3 zero cols each side; z[dt]: (128, L)
            xpad = []
            zt = []
            for dt in range(NDT):
                xp_ = mid.tile([P, L + 6], BF, tag=f"xpad{dt}_{rp2}")
                vec.memset(xp_[:, 0:3], 0.0)
                vec.memset(xp_[:, L + 3:L + 6], 0.0)
                xpad.append(xp_)
                zt.append(mid.tile([P, L], BF, tag=f"z{dt}_{rp2}", name=f"z{dt}"))
            for m in range(4):
                for n in range(2):
                    ps = psum.tile([P, 512], F32, tag="ps_main", name="ps")
                    for kt in range(4):
                        pe.matmul(ps[:], ipT[kt][:, m * 128:(m + 1) * 128],
                                  res[kt][:, n * 512:(n + 1) * 512],
                                  start=(kt == 0), stop=False)
                    pe.matmul(ps[:], augTs[:, m * 128:(m + 1) * 128],
                              aug[:, n * 512:(n + 1) * 512], start=False, stop=True)
                    if m < 2:
                        dst = xpad[m][:, 3 + n * 512: 3 + (n + 1) * 512]
                    else:
                        dst = zt[m - 2][:, n * 512:(n + 1) * 512]
                    vec.tensor_mul(dst, ps[:], rbc[:, n * 512:(n + 1) * 512])

            if upto == 'inproj':
                dbg_out(res[0])
            fctx.close()
            out_st["xpad"] = xpad
            out_st["zt"] = zt

        def phase_B1(rep, st):
            """Depthwise conv + silu -> u; x_dbl projection; AllReduce."""
            rp2 = rep % 2
            xdbl_loc, xdbl_red = scr[rp2]["xdbl_loc"], scr[rp2]["xdbl_red"]
            xpad = st["xpad"]
            u_cat = [mid.tile([P, T2], BF, tag=f"u{dt}_{rp2}", name=f"u{dt}")
                     for dt in range(NDT)]
            st["u_cat"] = u_cat
            # Depthwise causal conv as 4 diag-matmul taps accumulated in PSUM
            # (fwd reads x[t-3+k] -> xpad offset ch*512+k; rev is the
            # anti-causal conv y[t] = sum_k w[k] x[t+3-k], written reversed
            # into the tau domain), then u = silu from PSUM.
            for dt in range(NDT):
                X = xpad[dt]
                for d in range(2):  # 0 = fwd, 1 = rev (tau domain)
                    for ch in range(2):
                        dps = psum.tile([P, 512], F32, tag="ps_main", name="dps")
                        for k in range(4):
                            off = (ch * 512 + k) if d == 0 else (ch * 512 + 6 - k)
                            pe.matmul(dps[:], cvwDs[d][dt][k][:],
                                      X[:, off:off + 512],
                                      start=(k == 0), stop=(k == 3))
                        sg = scanp.tile([P, 512], BF, tag="dwsg", bufs=2)
                        act.activation(sg[:], dps[:], ActFn.Sigmoid,
                                       bias=cvbs[d][dt][:])
                        if d == 0:
                            dst = u_cat[dt][:, ch * 512:(ch + 1) * 512]
                        else:
                            dst = u_cat[dt][:, T2 - 1 - ch * 512:
                                           T2 - 513 - ch * 512:-1]
                        vec.scalar_tensor_tensor(dst, dps[:], cvbs[d][dt][:], sg[:],
                                                 AluOp.add, AluOp.mult)

            zs_t = [mid.tile([P, L], BF, tag=f"zs{dt}_{rp2}", name=f"zs{dt}")
                    for dt in range(NDT)]
            st["zs"] = zs_t
            for dt in range(NDT):
                sgz = scanp.tile([P, L], BF, tag="sgz", bufs=1)
                act.activation(sgz[:], st["zt"][dt][:], ActFn.Sigmoid)
                vec.tensor_mul(zs_t[dt][:], st["zt"][dt][:], sgz[:])

            if upto == 'dw':
                dbg_out(u_cat[0][:, 0:L])
                return
            # ---------------- x_dbl projection + AllReduce (bf16) ----------------
            for d in range(2):
                xsb = onep.tile([64, L], BF, tag="xsb")
                for n in range(2):
                    xps = psum.tile([64, 512], F32, tag="ps_main", name="xps")
                    for dt in range(NDT):
                        pe.matmul(xps[:], xpTs[d][dt][:],
                                  u_cat[dt][:, d * L + n * 512: d * L + (n + 1) * 512],
                                  start=(dt == 0), stop=(dt == 1))
                    act.activation(xsb[:, n * 512:(n + 1) * 512], xps[:], ActFn.Copy)
                sync.dma_start(xdbl_loc[d], xsb[:])
            pool.collective_compute(
                "AllReduce", AluOp.add,
                replica_groups=[[0, 1, 2, 3], [4, 5, 6, 7]],
                ins=[xdbl_loc[:].opt()],
                outs=[xdbl_red[:].opt()],
            )

        def phase_B2(rep, st):
            """dt_proj, selective scan, gate, out_proj, ReduceScatter."""
            rp2 = rep % 2
            xdbl_red = scr[rp2]["xdbl_red"]
            out_loc, out_rs = scr[rp2]["out_loc"], scr[rp2]["out_rs"]
            u_cat = st["u_cat"]
            if upto in ('dw', 'xdbl'):
                if upto == 'xdbl':
                    dbg_out(u_cat[0][:, 0:L])
                return
            # ---------------- dt_proj -> m = -softplus = ln(sigmoid(-x)) --------
            m_cat = [mid.tile([P, T2], BF, tag=f"m{dt}", name=f"m{dt}") for dt in range(NDT)]
            for d in range(2):
                dtfb = scanp.tile([32, L], BF, tag="dtfb", bufs=1)
                sync.dma_start(dtfb[:], xdbl_red[d, 0:32, :])
                for dt in range(NDT):
                    for n in range(2):
                        ps = psum.tile([P, 512], F32, tag="ps_main", name="ps")
                        pe.matmul(ps[:], dtTs[d][:, dt * 128:(dt + 1) * 128],
                                  dtfb[:, n * 512:(n + 1) * 512])
                        sgm = scanp.tile([P, 512], F32, tag="sgm")
                        act.activation(sgm[:], ps[:], ActFn.Sigmoid, scale=-1.0,
                                       bias=ndtbs[d][dt][:])
                        act.activation(m_cat[dt][:, d * L + n * 512: d * L + (n + 1) * 512],
                                       sgm[:], ActFn.Ln)

            # ux = -(m * u) = delta * u; then poison m[,0]/m[,L] so exp -> 0
            ux = [mid.tile([P, T2], BF, tag=f"ux{dt}", name=f"ux{dt}") for dt in range(NDT)]
            for dt in range(NDT):
                vec.scalar_tensor_tensor(ux[dt][:], m_cat[dt][:], -1.0, u_cat[dt][:],
                                         AluOp.mult, AluOp.mult)
            for dt in range(NDT):
                vec.memset(m_cat[dt][:, 0:1], -30.0)
                vec.memset(m_cat[dt][:, L:L + 1], -30.0)

            if upto == 'dt':
                dbg_out(m_cat[0][:, 0:L])
                return
            yield
            # ---------------- selective scan ----------------
            # Direction-outer: B_s/C_s broadcast once per (d, s) and shared by
            # both d-tiles; each (d, dt) accumulates y into a 2-bank PSUM half.
            # B_s rows live at xdbl_red[d, 32+s, :], C_s at xdbl_red[d, 48+s, :]
            xr_ap = xdbl_red[:]
            yc = {}
            ygate = []
            for d in range(2):
                ps_d = [psum.tile([P, L], F32, tag="ps_yd", name=f"ps_{d}{dt}",
                                  bufs=2) for dt in range(NDT)]
                for sp in range(8):  # s-pair (2*sp, 2*sp+1)
                    if sp in (2, 4, 6):
                        yield
                    s0 = 2 * sp
                    # one DMA per (d, s-pair): [B_s|B_s+1|C_s|C_s+1]; row pairs
                    # are contiguous in xdbl_red so descriptors stay simple
                    if PROBE == 'nobcast' and sp > 0:
                        bc = bc_prev
                    else:
                        bc = scanp.tile([P, 4 * L], BF, tag="Bs", bufs=2)
                        sync.dma_start(
                            bc[:],
                            _ap_bcast_dram(xr_ap.tensor,
                                           xr_ap.offset + (d * 64 + 32 + s0) * L,
                                           [[0, P], [16 * L, 2], [1, 2 * L]]),
                        )
                        bc_prev = bc
                    for dt in range(NDT):
                        sl = slice(d * L, (d + 1) * L)
                        a_s = scanp.tile([P, 2 * L], BF, tag="a_s")
                        act.activation(a_s[:, 0:L], m_cat[dt][:, sl], ActFn.Exp,
                                       scale=float(-a_vals[s0]))
                        act.activation(a_s[:, L:2 * L], m_cat[dt][:, sl], ActFn.Exp,
                                       scale=float(-a_vals[s0 + 1]))
                        # ux repeated across the pair via a stride-0 middle dim
                        uxh = ux[dt][:, sl]
                        ux2 = bass.AP(tensor=uxh.tensor, offset=uxh.offset,
                                      ap=[list(uxh.ap[0]), [0, 2], [1, L]])
                        b_s = scanp.tile([P, 2 * L], BF, tag="b_s")
                        bp = (dt == 0)
                        gp = (dt == 1 and sp % 2 == 1)
                        (pool if bp else vec).tensor_mul(b_s[:], ux2, bc[:, 0:2 * L])
                        h_s = scanp.tile([P, 2 * L], BF, tag="h_s")
                        if PROBE == 'noscan':
                            vec.tensor_mul(h_s[:], a_s[:], b_s[:])
                        else:
                            vec.tensor_tensor_scan(h_s[:], a_s[:], b_s[:], 0.0,
                                                   AluOp.mult, AluOp.add)
                        gs = scanp.tile([P, 2 * L], BF, tag="b_s", name="gs")
                        (pool if gp else vec).tensor_mul(gs[:], h_s[:],
                                                         bc[:, 2 * L:4 * L])
                        for j in range(4):
                            pe.matmul(ps_d[dt][:, (j % 2) * 512:(j % 2) * 512 + 512],
                                      ident[:], gs[:, j * 512:(j + 1) * 512],
                                      start=(sp == 0 and j < 2), stop=False)
                # D-term (diag D matmul on u) closes each accumulation
                for dt in range(NDT):
                    for ch in range(2):
                        pe.matmul(ps_d[dt][:, ch * 512:(ch + 1) * 512], dDs[d][dt][:],
                                  u_cat[dt][:, d * L + ch * 512: d * L + (ch + 1) * 512],
                                  start=False, stop=True)
                    t = onep.tile([P, L], BF, tag=f"yc{d}{dt}", name=f"yc{d}{dt}")
                    act.activation(t[:], ps_d[dt][:], ActFn.Copy)
                    yc[(d, dt)] = t
                if d == 0:
                    yield
            for dt in range(NDT):
                ysum = onep.tile([P, L], BF, tag="ysum")
                vec.tensor_add(ysum[:], yc[(0, dt)][:],
                               yc[(1, dt)][:, L - 1::-1])
                yg = scanp.tile([P, L], BF, tag="yg")
                vec.tensor_mul(yg[:], ysum[:], st["zs"][dt][:])
                ygate.append(yg)

            if upto == 'scan':
                dbg_out(yc[(0, 0)])
                return

            # ---------------- out_proj + res/4 -> bf16 ReduceScatter ------------
            rd = res_dram[st["rp3"]]
            for m in range(4):
                osb = onep.tile([P, L], BF, tag="osb", bufs=2)
                resc = scanp.tile([P, L], BF, tag="resc", bufs=1)
                sync.dma_start(resc[:], rd[m * 128:(m + 1) * 128, :])
                for n in range(2):
                    ps = psum.tile([P, 512], F32, tag="ps_main", name="ps")
                    for dt in range(NDT):
                        pe.matmul(ps[:], outTs[dt][:, m * 128:(m + 1) * 128],
                                  ygate[dt][:, n * 512:(n + 1) * 512],
                                  start=(dt == 0), stop=(dt == 1))
                    vec.scalar_tensor_tensor(osb[:, n * 512:(n + 1) * 512],
                                             resc[:, n * 512:(n + 1) * 512],
                                             1.0 / NGRP, ps[:],
                                             AluOp.mult, AluOp.add)
                sync.dma_start(out_loc[m * 128:(m + 1) * 128, :], osb[:])
            pool.collective_compute(
                "ReduceScatter", AluOp.add,
                replica_groups=[[0, 1, 2, 3], [4, 5, 6, 7]],
                ins=[out_loc[:].opt()],
                outs=[out_rs[:].opt()],
            )
            sync.dma_start(out_ext[:], out_rs[:])

        # Skewed emission: A(k) | B1(k-1) | B2(k-2).  In-order engines then
        # interleave rep k's stem (PE/Act) with rep k-2's scan (DVE/Pool),
        # and rep k-1's AllReduce overlaps both.
        def step(g):
            if g is None:
                return False
            try:
                next(g)
                return True
            except StopIteration:
                return False

        states = {}
        if upto in ('stem', 'inproj'):
            for k in range(reps):
                states[k] = {}
                for _ in phase_A(k, states[k]):
                    pass
        else:
            for k in range(reps + 2):
                gb = phase_B2(k - 2, states.pop(k - 2)) if 2 <= k <= reps + 1 else None
                ga = None
                if k < reps:
                    states[k] = {}
                    ga = phase_A(k, states[k])
                step(gb); step(ga); step(gb)
                if 1 <= k <= reps:
                    phase_B1(k - 1, states[k - 1])
                alive = True
                while alive:
                    alive = step(gb)
                    alive = step(ga) or alive

    if split_waits:
        split_excess_waits(nc)
    return nc


def prep_inputs(inputs):
    """Host-side sharding/weight prep.  Returns (a_vals, in_maps)."""
    f32 = lambda a: np.ascontiguousarray(np.asarray(a, np.float32))
    bf = lambda a: np.ascontiguousarray(np.asarray(a, np.float32).astype(BF16))

    A_f = -np.exp(f32(inputs["Alog_f"]))
    A_r = -np.exp(f32(inputs["Alog_r"]))
    assert np.abs(A_f - A_f[0:1]).max() < 1e-5, "A not d-independent"
    assert np.abs(A_f - A_r).max() < 1e-5, "A_f != A_r"
    a_vals = [float(v) for v in A_f[0]]

    x = f32(inputs["x"])
    w1 = f32(inputs["conv1_w"]); w2 = f32(inputs["conv2_w"]); w3 = f32(inputs["conv3_w"])
    w1T = bf(np.transpose(w1, (2, 1, 0)).reshape(3, 6, 128, 128))
    w2T = bf(np.transpose(w2, (2, 1, 0)).reshape(3, 1, 128, 256))
    w3T = bf(np.transpose(w3, (2, 1, 0)).reshape(3, 2, 128, 512))
    onehot = np.zeros((3, 128, 32), np.float32)
    for i, cg in enumerate((4, 8, 16)):
        onehot[i, np.arange(128), np.arange(128) // cg] = 1.0
    ln_g = f32(inputs["ln_g"]); ln_b = f32(inputs["ln_b"])
    ipw = f32(inputs["in_proj_w"])
    opw = f32(inputs["out_proj_w"])

    common = dict(
        w1T=w1T, w2T=w2T, w3T=w3T,
        cb1=f32(inputs["conv1_b"]).reshape(128, 1),
        cb2=f32(inputs["conv2_b"]).reshape(256, 1),
        cb3=f32(inputs["conv3_b"]).reshape(512, 1),
        gng1=f32(inputs["gn1_g"]).reshape(128, 1),
        gnb1=f32(inputs["gn1_b"]).reshape(128, 1),
        gng2=f32(inputs["gn2_g"]).reshape(256, 1),
        gnb2=f32(inputs["gn2_b"]).reshape(256, 1),
        gng3=f32(inputs["gn3_g"]).reshape(512, 1),
        gnb3=f32(inputs["gn3_b"]).reshape(512, 1),
        onehot=bf(onehot),
        onehotE=bf(np.transpose(onehot, (0, 2, 1))),
        ones_col=bf(np.ones((128, 1), np.float32)),
        identw=bf(np.eye(128, dtype=np.float32)),
    )

    in_maps = []
    for core in range(NCORES):
        b, grp = core // NGRP, core % NGRP
        rows = np.arange(grp * DSH, (grp + 1) * DSH)
        sel = np.concatenate([rows, DI + rows])
        Wsel = ipw[sel] * ln_g[None, :]
        inprojT = bf(Wsel.T.reshape(4, 128, 2 * DSH))
        augTm = bf(np.stack([Wsel.sum(1), ipw[sel] @ ln_b]))
        xpTm = np.stack([
            bf(f32(inputs[f"xp_w_{s}"])[:, rows].T.reshape(2, 128, 64))
            for s in ("f", "r")])
        dtTm = np.stack([
            bf(f32(inputs[f"dt_w_{s}"])[rows].T) for s in ("f", "r")])
        ndtbm = np.stack([
            -f32(inputs[f"dt_b_{s}"])[rows].reshape(DSH, 1) for s in ("f", "r")])
        cvwDm = np.zeros((2, NDT, DC, 128, 128), np.float32)
        for di, s in enumerate(("f", "r")):
            wv = f32(inputs[f"cv_w_{s}"])[rows, 0]          # (DSH, 4)
            for dt in range(NDT):
                for k in range(DC):
                    np.fill_diagonal(cvwDm[di, dt, k], wv[dt * 128:(dt + 1) * 128, k])
        cvbm = np.stack([
            f32(inputs[f"cv_b_{s}"])[rows].reshape(DSH, 1) for s in ("f", "r")])
        dDm = np.zeros((2, NDT, 128, 128), np.float32)
        for di, s in enumerate(("f", "r")):
            Dv = f32(inputs[f"D_{s}"])[rows]
            for dt in range(NDT):
                np.fill_diagonal(dDm[di, dt], Dv[dt * 128:(dt + 1) * 128])
        outTm = bf(opw[:, rows].T.reshape(2, 128, DM))
        xpadded = bf(np.pad(x[b], ((0, 0), (1, 1))))
        m = dict(common)
        m.update(x=xpadded, inprojT=inprojT, augT=augTm, xpT=xpTm, dtT=dtTm,
                 ndtb=ndtbm, cvwD=bf(cvwDm), cvb=cvbm, dDiag=bf(dDm), outT=outTm)
        in_maps.append(m)
    return a_vals, in_maps


def assemble_output(results):
    """results: list of 8 dicts with 'out' (128, L).  Each group of 4 cores
    holds the 4 row-quarters of its batch sample."""
    full = []
    for b in range(B):
        full.append(np.concatenate(
            [np.asarray(results[b * NGRP + g]["out"], np.float32)
             for g in range(NGRP)], axis=0))
    return np.ascontiguousarray(np.stack(full).astype(np.float32))


def kernel(**inputs) -> np.ndarray:
    from concourse.bass_utils import run_bass_kernel_spmd
    a_vals, in_maps = prep_inputs(inputs)
    nc = build_program(a_vals)
    res = run_bass_kernel_spmd(nc, in_maps, list(range(NCORES)))
    return assemble_output(res.results)


if __name__ == "__main__":
    import reference as R
    import jax
    with jax.default_device(jax.devices("cpu")[0]):
        inp = {k: np.asarray(v) for k, v in R.setup_inputs().items()}
        ref = np.asarray(R.reference(**R.setup_inputs()))
    got = kernel(**inp)
    err = np.abs(got - ref).max() / np.abs(ref).max()
    print("Relative error:", err)


# revision 33
# speedup vs baseline: 1.6329x; 1.6329x over previous
"""Trainium2 Bass kernel for nn_CLIP_Embedding_35613868818658.

CNN stem (3x conv1d+GroupNorm+ReLU, 768->128->256->512) -> LayerNorm ->
bidirectional Mamba (selective scan, d_inner=1024, d_state=16, L=1024) ->
out_proj + residual.  Output (2, 512, 1024) f32.

Sharding: 2 batch-groups x 4-way d_inner split (DSH=256 rows per core).
Cores 0-3 handle b=0, cores 4-7 handle b=1; core g within a group owns
d_inner rows [256g, 256(g+1)).  Two in-group collectives, both bf16:
AllReduce of x_dbl (B/C/dt projections, contracted over sharded d_inner)
and a ReduceScatter of out_proj partials (+res/4), so each core emits its
own 128-row quarter of the output and the host concatenates.

The selective scan runs as 16 (one per state index s) tensor_tensor_scan
instructions per d-tile over a [128, 2048] layout that concatenates the
forward and (time-reversed) backward directions along the free axis;
m[, t=0|1024] = -30 makes a = exp((s+1)m) ~ 0, resetting the recurrence at
segment starts.  The s-contraction y = sum_s C_s*h_s runs on the PE as
PSUM-accumulated identity matmuls (plus diag(D) matmuls for the D-term);
scans are split DVE/Pool to balance engine load.
"""

import numpy as np
import ml_dtypes

import concourse.bass as bass
import concourse.mybir as mybir
import concourse.tile as tile
from contextlib import ExitStack

BF16 = ml_dtypes.bfloat16
F32 = mybir.dt.float32
BF = mybir.dt.bfloat16

B, CIN, L = 2, 768, 1024
DM, DI, DS, DTR, DC = 512, 1024, 16, 32, 4
NCORES, NGRP = 8, 4
DSH = DI // NGRP          # 256 d_inner rows per core
NDT = DSH // 128          # 2 d-tiles of 128 partitions
T2 = 2 * L                # fwd|rev concatenated time axis
EPS = 1e-5
# walrus only lowers tensor_tensor_scan on DVE; plain TensorTensor mults are
# Pool-legal, so engine balance comes from sending ~45% of the b/g multiplies
# to Pool (Pool mult is ~3.7x slower than DVE's 2x-mode mult).
POOL_BS = frozenset((1, 3, 5, 7, 9, 11, 13))
POOL_GS = frozenset((0, 2, 4, 6, 8, 10, 12))

# timing-only probes (wrong numerics): 'nobcast' hoists the B/C broadcast
# out of the s-loop; 'noscan' replaces tensor_tensor_scan with a plain mult
PROBE = None
POOL_MULTS = False

AluOp = mybir.AluOpType
ActFn = mybir.ActivationFunctionType


def _ap_bcast_dram(handle, offset, dims):
    """Raw AP on a DRAM tensor: dims is a list of [step, count]."""
    return bass.AP(tensor=handle, offset=offset, ap=[list(d) for d in dims])


def split_excess_waits(nc, max_waits=1):
    """Walrus rejects instructions carrying more sync waits than the ISA
    encoding has slots for (1 on this toolchain).  Move excess waits onto
    preceding same-engine NoOps."""
    for bb in nc.main_func.blocks:
        insts = bb.instructions
        out, changed = [], False
        for ins in insts:
            si = ins.sync_info
            if si is not None and si.on_wait is not None and len(si.on_wait) > max_waits:
                waits = list(si.on_wait)
                keep, rest = waits[:max_waits], waits[max_waits:]
                idx = 0
                while rest:
                    chunk, rest = rest[:max_waits], rest[max_waits:]
                    nop = mybir.InstNoOp(
                        name=f"{ins.name}-wsplit{idx}",
                        engine=ins.engine,
                        sync_info=mybir.SyncInfo(on_wait=chunk, on_update=[]),
                        bass_nofuse=True,
                    )
                    out.append(nop)
                    idx += 1
                ins.sync_info = mybir.SyncInfo(
                    on_wait=keep, on_update=list(si.on_update or [])
                )
                changed = True
            out.append(ins)
        if changed:
            bb.instructions = out
    return nc


def build_program(a_vals, split_waits=True, debug_dumps=False, reps=1, upto='full'):
    """Build the SPMD Bass program.  a_vals: 16 negative floats, A[s] = -(s+1)
    (verified d-independent and equal for both directions on the host)."""
    nc = bass.Bass("TRN2", target_bir_lowering=False, debug=False,
                   num_devices=NCORES)

    dt_in = lambda n, s, d=BF: nc.dram_tensor(n, list(s), d, kind="ExternalInput")

    x_in = dt_in("x", (CIN, L + 2))                      # host-padded, bf16
    w1T = dt_in("w1T", (3, 6, 128, 128))
    w2T = dt_in("w2T", (3, 1, 128, 256))
    w3T = dt_in("w3T", (3, 2, 128, 512))
    cb1 = dt_in("cb1", (128, 1), F32)
    cb2 = dt_in("cb2", (256, 1), F32)
    cb3 = dt_in("cb3", (512, 1), F32)
    gng1 = dt_in("gng1", (128, 1), F32)
    gnb1 = dt_in("gnb1", (128, 1), F32)
    gng2 = dt_in("gng2", (256, 1), F32)
    gnb2 = dt_in("gnb2", (256, 1), F32)
    gng3 = dt_in("gng3", (512, 1), F32)
    gnb3 = dt_in("gnb3", (512, 1), F32)
    onehot = dt_in("onehot", (3, 128, 32))
    onehotE = dt_in("onehotE", (3, 32, 128))
    ones_col = dt_in("ones_col", (128, 1))
    inprojT = dt_in("inprojT", (4, 128, 512))
    augT = dt_in("augT", (2, 512))
    xpT = dt_in("xpT", (2, 2, 128, 64))                 # [dir][ktile]
    dtT = dt_in("dtT", (2, 32, 256))                    # [dir]
    ndtb = dt_in("ndtb", (2, 256, 1), F32)              # -dt_b
    cvwD = dt_in("cvwD", (2, 2, 4, 128, 128))          # [dir][dtile][k] diag
    cvb = dt_in("cvb", (2, 256, 1), F32)
    outT = dt_in("outT", (2, 128, 512))                 # [dtile]
    identw = dt_in("identw", (128, 128))
    idq_w = dt_in("idq_w", (128, 128))                  # identity / NGRP
    dDiag = dt_in("dDiag", (2, 2, 128, 128))            # [dir][dtile] diag(D)

    out_ext = nc.dram_tensor("out", [128, L], BF, kind="ExternalOutput")

    with tile.TileContext(nc) as tc, ExitStack() as ctx:
        P = 128
        consts = ctx.enter_context(tc.tile_pool(name="consts", bufs=1))
        psum = ctx.enter_context(tc.tile_pool(name="psum", bufs=2, space="PSUM"))
        mid = ctx.enter_context(tc.tile_pool(name="mid", bufs=1))
        dram = ctx.enter_context(tc.tile_pool(name="dram", bufs=1, space="DRAM"))
        sync, vec, pool, act, pe = nc.sync, nc.vector, nc.gpsimd, nc.scalar, nc.tensor

        # ---------------- consts to SBUF ----------------
        def load(poolh, shape, src, dtype=BF, name=None):
            t = poolh.tile(list(shape), dtype, tag=name)
            sync.dma_start(t[:], src)
            return t

        w1 = [[load(consts, (P, 128), w1T[k, ct], name=f"w1_{k}_{ct}")
               for ct in range(6)] for k in range(3)]
        w2 = [[load(consts, (P, 256), w2T[k, ct], name=f"w2_{k}_{ct}")
               for ct in range(1)] for k in range(3)]
        w3 = [[load(consts, (P, 512), w3T[k, ct], name=f"w3_{k}_{ct}")
               for ct in range(2)] for k in range(3)]
        def load_cols(dramt, co, name, width=1):
            return [load(consts, (128, width), dramt[mt * 128:(mt + 1) * 128, :],
                         F32, f"{name}{mt}") for mt in range(co // 128)]

        cbs = [load_cols(cb1, 128, "cb1"), load_cols(cb2, 256, "cb2"),
               load_cols(cb3, 512, "cb3")]
        gngs = [load_cols(gng1, 128, "gng1"), load_cols(gng2, 256, "gng2"),
                load_cols(gng3, 512, "gng3")]
        gnbs = [load_cols(gnb1, 128, "gnb1"), load_cols(gnb2, 256, "gnb2"),
                load_cols(gnb3, 512, "gnb3")]
        oneh = [load(consts, (P, 32), onehot[i], name=f"onehot{i}")
                for i in range(3)]
        onehE = [load(consts, (32, 128), onehotE[i], name=f"onehotE{i}")
                 for i in range(3)]
        ones1 = load(consts, (P, 1), ones_col[:], name="ones1")
        ipT = [load(consts, (P, 512), inprojT[kt], name=f"ipT{kt}") for kt in range(4)]
        augTs = load(consts, (2, 512), augT[:], name="augT")
        xpTs = [[load(consts, (P, 64), xpT[d, kt], name=f"xpT{d}{kt}")
                 for kt in range(2)] for d in range(2)]
        dtTs = [load(consts, (32, 256), dtT[d], name=f"dtT{d}") for d in range(2)]
        ndtbs = [[load(consts, (128, 1), ndtb[d, dt * 128:(dt + 1) * 128, :], F32,
                       f"ndtb{d}{dt}") for dt in range(2)] for d in range(2)]
        cvwDs = [[[load(consts, (P, 128), cvwD[d, dt, k], name=f"cvwD{d}{dt}{k}")
                   for k in range(4)] for dt in range(2)] for d in range(2)]
        cvbs = [[load(consts, (128, 1), cvb[d, dt * 128:(dt + 1) * 128, :], F32,
                      f"cvb{d}{dt}") for dt in range(2)] for d in range(2)]
        outTs = [load(consts, (P, 512), outT[dt], name=f"outT{dt}") for dt in range(2)]
        ident = load(consts, (P, 128), identw[:], name="ident")
        idq = load(consts, (P, 128), idq_w[:], name="idq")
        dDs = [[load(consts, (P, 128), dDiag[d, dt], name=f"dD{d}{dt}")
                for dt in range(2)] for d in range(2)]

        epsc = consts.tile([128, 1], F32, tag="epsc")
        vec.memset(epsc[:], EPS)

        # res spill: written by A(rep), read back by B2(rep) two rounds later
        res_dram = [dram.tile([DM, L], BF, tag=f"res_dram{r}", name="g")
                    for r in range(3)]
        # DRAM scratch, double-buffered so rep i+1 overlaps rep i
        scr = []
        for rp in range(2):
            scr.append(dict(
                ln_scr=dram.tile([1, L], BF, tag=f"ln_scr{rp}", name="g"),
                xdbl_loc=dram.tile([2, 64, L], BF, tag=f"xdbl_loc{rp}", name="g"),
                xdbl_red=dram.tile([2, 64, L], BF, tag=f"xdbl_red{rp}", name="g"),
                out_loc=dram.tile([DM, L], BF, tag=f"out_loc{rp}", name="g"),
                out_rs=dram.tile([128, L], BF, tag=f"out_rs{rp}", name="g"),
            ))

        def dbg_out(src_ap):
            t = mid.tile([P, L], BF, tag="dbg_cast", name="dbg_cast")
            act.activation(t[:], src_ap if isinstance(src_ap, bass.AP) else src_ap[:],
                           ActFn.Copy)
            sync.dma_start(out_ext[:], t[:])

        scanp = ctx.enter_context(tc.tile_pool(name="scanp", bufs=2))
        onep = ctx.enter_context(tc.tile_pool(name="onep", bufs=1))

        def phase_A(rep, out_st):
            """Stem + LayerNorm + in_proj (generator: yields between conv
            layers so the driver can interleave emission with the scan of an
            earlier rep).  Writes res_dram (rep%3), xpad/z (rep%2)."""
            rp2, rp3 = rep % 2, rep % 3
            ln_scr = scr[rep % 2]["ln_scr"]
            fctx = ExitStack()
            stem = fctx.enter_context(tc.tile_pool(name=f"stem{rep}", bufs=1))
            stemtmp = fctx.enter_context(tc.tile_pool(name=f"stemtmp{rep}", bufs=2))
            statp = fctx.enter_context(tc.tile_pool(name=f"statp{rep}", bufs=2))
            rows = fctx.enter_context(tc.tile_pool(name=f"rows{rep}", bufs=1))
            x_t = [load(stem, (P, L + 2), x_in[ct * P:(ct + 1) * P, :],
                        name=f"x{ct}") for ct in range(6)]
            # ---------------- CNN stem ----------------
            def conv_gn_relu(layer, in_tiles, ws, cb, gng, gnb, co, to_mid):
                # generator: yields after each 128-channel tile
                """in_tiles: list of padded (128, L+2) bf16; returns list of
                normalized+relu'd output tiles.  to_mid: final layer (res)."""
                n_ct = len(in_tiles)
                n_co = co // 128
                cg = co // 32            # channels per group
                ngt = 128 // cg          # groups per 128-channel tile
                group_elems = float(cg) * L
                outs = []
                for mt in range(n_co):
                    h_raw = stemtmp.tile([P, L], F32, tag="h_raw")
                    stat4 = statp.tile([P, 4], F32, tag="stat4")
                    sq = stemtmp.tile([P, 512], BF, tag="sq", bufs=1)
                    for n in range(2):
                        ps = psum.tile([P, 512], F32, tag="ps_main", name="ps")
                        nmm = n_ct * 3
                        i = 0
                        for ct in range(n_ct):
                            for k in range(3):
                                pe.matmul(
                                    ps[:],
                                    ws[k][ct][:, mt * 128:(mt + 1) * 128],
                                    in_tiles[ct][:, n * 512 + k: n * 512 + k + 512],
                                    start=(i == 0), stop=(i == nmm - 1),
                                )
                                i += 1
                        act.activation(h_raw[:, n * 512:(n + 1) * 512], ps[:],
                                       ActFn.Identity, bias=cb[mt][:],
                                       accum_out=stat4[:, n:n + 1])
                        act.activation(sq[:], h_raw[:, n * 512:(n + 1) * 512],
                                       ActFn.Square, accum_out=stat4[:, 2 + n:3 + n])
                    # group stats: per-partition sums -> per-group via one-hot matmul
                    stat4b = statp.tile([P, 4], BF, tag="stat4b")
                    vec.tensor_copy(stat4b[:], stat4[:])
                    gps = psum.tile([32, 4], F32, tag="ps_row", name="gps", bufs=2)
                    pe.matmul(gps[:], oneh[layer - 1][:], stat4b[:])
                    gsb = statp.tile([32, 4], F32, tag="gsb")
                    act.activation(gsb[:], gps[:], ActFn.Copy)
                    sx = statp.tile([32, 1], F32, tag="sx")
                    sq_g = statp.tile([32, 1], F32, tag="sq_g")
                    vec.tensor_add(sx[:], gsb[:, 0:1], gsb[:, 1:2])
                    vec.tensor_add(sq_g[:], gsb[:, 2:3], gsb[:, 3:4])
                    mean = statp.tile([32, 1], F32, tag="mean")
                    act.activation(mean[:], sx[:], ActFn.Copy, scale=1.0 / group_elems)
                    msq = statp.tile([32, 1], F32, tag="msq")
                    act.activation(msq[:], sx[:], ActFn.Square, scale=1.0 / group_elems)
                    var = statp.tile([32, 1], F32, tag="var")
                    vec.scalar_tensor_tensor(var[:], sq_g[:], 1.0 / group_elems, msq[:],
                                             AluOp.mult, AluOp.subtract)
                    sig_g = statp.tile([32, 1], F32, tag="sig_g")
                    act.activation(sig_g[:], var[:], ActFn.Sqrt, bias=epsc[:32, :])
                    rstd = statp.tile([32, 1], F32, tag="rstd")
                    vec.reciprocal(rstd[:], sig_g[:])
                    # pack [rstd, mean]; expand groups 32 -> channels 128 via
                    # a one-hot matmul (no DRAM round-trip)
                    stat2 = statp.tile([32, 2], BF, tag="stat2")
                    vec.tensor_copy(stat2[:, 0:1], rstd[:])
                    vec.tensor_copy(stat2[:, 1:2], mean[:])
                    ch2p = psum.tile([P, 2], F32, tag="ps_row", name="ch2p", bufs=2)
                    pe.matmul(ch2p[:], onehE[layer - 1][:], stat2[:])
                    scale_c = statp.tile([P, 1], F32, tag="scale_c")
                    vec.tensor_mul(scale_c[:], ch2p[:, 0:1], gng[mt][:])
                    nmean_s = statp.tile([P, 1], F32, tag="nmean_s")
                    vec.tensor_mul(nmean_s[:], ch2p[:, 1:2], scale_c[:])
                    bias_c = statp.tile([P, 1], F32, tag="bias_c")
                    vec.tensor_sub(bias_c[:], gnb[mt][:], nmean_s[:])
                    if to_mid:
                        h_out = stem.tile([P, L], BF, tag=f"res{mt}")
                        act.activation(h_out[:], h_raw[:], ActFn.Relu,
                                       scale=scale_c[:], bias=bias_c[:])
                        sync.dma_start(res_dram[rp3][mt * 128:(mt + 1) * 128, :],
                                       h_out[:])
                    else:
                        h_out = stem.tile([P, L + 2], BF, tag=f"h{layer}_{mt}")
                        vec.memset(h_out[:, 0:1], 0.0)
                        vec.memset(h_out[:, L + 1:L + 2], 0.0)
                        act.activation(h_out[:, 1:L + 1], h_raw[:], ActFn.Relu,
                                       scale=scale_c[:], bias=bias_c[:])
                    outs.append(h_out)
                    yield
                return outs

            h1 = yield from conv_gn_relu(1, x_t, w1, cbs[0], gngs[0], gnbs[0], 128, False)
            h2 = yield from conv_gn_relu(2, h1, w2, cbs[1], gngs[1], gnbs[1], 256, False)
            res = yield from conv_gn_relu(3, h2, w3, cbs[2], gngs[2], gnbs[2], 512, True)
            out_st["rp3"] = rp3
            yield

            if upto == 'stem':
                dbg_out(res[0])
                fctx.close()
                return
            # ---------------- LayerNorm stats (over channels, via matmuls) -------
            hsq = []
            for mt in range(4):
                t = stemtmp.tile([P, L], BF, tag="hsq")
                act.activation(t[:], res[mt][:], ActFn.Square)
                hsq.append(t)
            musum = rows.tile([1, L], BF, tag="musum")
            sqsum = rows.tile([1, L], BF, tag="sqsum")
            for n in range(2):
                mu_ps = psum.tile([1, 512], F32, tag="ps_row", name="mu_ps", bufs=2)
                for kt in range(4):
                    pe.matmul(mu_ps[:], ones1[:],
                              res[kt][:, n * 512:(n + 1) * 512],
                              start=(kt == 0), stop=(kt == 3))
                act.activation(musum[:, n * 512:(n + 1) * 512], mu_ps[:], ActFn.Copy)
                sq_ps = psum.tile([1, 512], F32, tag="ps_row", name="sq_ps", bufs=2)
                for kt in range(4):
                    pe.matmul(sq_ps[:], ones1[:],
                              hsq[kt][:, n * 512:(n + 1) * 512],
                              start=(kt == 0), stop=(kt == 3))
                act.activation(sqsum[:, n * 512:(n + 1) * 512], sq_ps[:], ActFn.Copy)
            msql = rows.tile([1, L], BF, tag="msql")
            act.activation(msql[:], musum[:], ActFn.Square, scale=1.0 / DM)
            varl = rows.tile([1, L], BF, tag="varl")
            vec.scalar_tensor_tensor(varl[:], sqsum[:], 1.0 / DM, msql[:],
                                     AluOp.mult, AluOp.subtract)
            sigma = rows.tile([1, L], BF, tag="sigma")
            act.activation(sigma[:], varl[:], ActFn.Sqrt, bias=epsc[:1, :])
            recip = rows.tile([1, L], BF, tag="msql", name="recip")
            with nc.allow_low_precision(reason="LN 1/sigma in bf16; |err|~4e-3 ok"):
                vec.reciprocal(recip[:], sigma[:])
            nmu_b = rows.tile([1, L], BF, tag="varl", name="nmu_b")
            act.activation(nmu_b[:], musum[:], ActFn.Identity, scale=-1.0 / DM)
            aug = rows.tile([2, L], BF, tag="aug")
            sync.dma_start(aug[0:1, :], nmu_b[:])
            sync.dma_start(aug[1:2, :], sigma[:])
            sync.dma_start(ln_scr[:], recip[:])
            rbc = rows.tile([P, L], BF, tag="rbc")
            sync.dma_start(
                rbc[:],
                _ap_bcast_dram(ln_scr[:].tensor, ln_scr[:].offset, [[0, P], [1, L]]),
            )

            # ---------------- in_proj (LN folded in) ----------------
            # xpad[dt]: (128, L+6) bf16, 3 zero cols each side; z[dt]: (128, L)
            xpad = []
            zt = []
            for dt in range(NDT):
                xp_ = mid.tile([P, L + 6], BF, tag=f"xpad{dt}_{rp2}")
                vec.memset(xp_[:, 0:3], 0.0)
                vec.memset(xp_[:, L + 3:L + 6], 0.0)
                xpad.append(xp_)
                zt.append(mid.tile([P, L], BF, tag=f"z{dt}_{rp2}", name=f"z{dt}"))
            for m in range(4):
                for n in range(2):
                    ps = psum.tile([P, 512], F32, tag="ps_main", name="ps")
                    for kt in range(4):
                        pe.matmul(ps[:], ipT[kt][:, m * 128:(m + 1) * 128],
                                  res[kt][:, n * 512:(n + 1) * 512],
                                  start=(kt == 0), stop=False)
                    pe.matmul(ps[:], augTs[:, m * 128:(m + 1) * 128],
                              aug[:, n * 512:(n + 1) * 512], start=False, stop=True)
                    if m < 2:
                        dst = xpad[m][:, 3 + n * 512: 3 + (n + 1) * 512]
                    else:
                        dst = zt[m - 2][:, n * 512:(n + 1) * 512]
                    vec.tensor_mul(dst, ps[:], rbc[:, n * 512:(n + 1) * 512])

            if upto == 'inproj':
                dbg_out(res[0])
            fctx.close()
            out_st["xpad"] = xpad
            out_st["zt"] = zt

        def phase_B1(rep, st):
            """Depthwise conv + silu -> u; x_dbl projection; AllReduce."""
            rp2 = rep % 2
            xdbl_loc, xdbl_red = scr[rp2]["xdbl_loc"], scr[rp2]["xdbl_red"]
            xpad = st["xpad"]
            u_cat = [mid.tile([P, T2], BF, tag=f"u{dt}_{rp2}", name=f"u{dt}")
                     for dt in range(NDT)]
            st["u_cat"] = u_cat
            # Depthwise causal conv as 4 diag-matmul taps accumulated in PSUM
            # (fwd reads x[t-3+k] -> xpad offset ch*512+k; rev is the
            # anti-causal conv y[t] = sum_k w[k] x[t+3-k], written reversed
            # into the tau domain), then u = silu from PSUM.
            for dt in range(NDT):
                X = xpad[dt]
                for d in range(2):  # 0 = fwd, 1 = rev (tau domain)
                    for ch in range(2):
                        dps = psum.tile([P, 512], F32, tag="ps_main", name="dps")
                        for k in range(4):
                            off = (ch * 512 + k) if d == 0 else (ch * 512 + 6 - k)
                            pe.matmul(dps[:], cvwDs[d][dt][k][:],
                                      X[:, off:off + 512],
                                      start=(k == 0), stop=(k == 3))
                        sg = scanp.tile([P, 512], BF, tag="dwsg", bufs=1)
                        act.activation(sg[:], dps[:], ActFn.Sigmoid,
                                       bias=cvbs[d][dt][:])
                        if d == 0:
                            dst = u_cat[dt][:, ch * 512:(ch + 1) * 512]
                        else:
                            dst = u_cat[dt][:, T2 - 1 - ch * 512:
                                           T2 - 513 - ch * 512:-1]
                        vec.scalar_tensor_tensor(dst, dps[:], cvbs[d][dt][:], sg[:],
                                                 AluOp.add, AluOp.mult)

            zs_t = [mid.tile([P, L], BF, tag=f"zs{dt}_{rp2}", name=f"zs{dt}")
                    for dt in range(NDT)]
            st["zs"] = zs_t
            for dt in range(NDT):
                sgz = scanp.tile([P, L], BF, tag="sgz", bufs=1)
                act.activation(sgz[:], st["zt"][dt][:], ActFn.Sigmoid)
                vec.tensor_mul(zs_t[dt][:], st["zt"][dt][:], sgz[:])

            if upto == 'dw':
                dbg_out(u_cat[0][:, 0:L])
                return
            # ---------------- x_dbl projection + AllReduce (bf16) ----------------
            for d in range(2):
                xsb = onep.tile([64, L], BF, tag="xsb")
                for n in range(2):
                    xps = psum.tile([64, 512], F32, tag="ps_main", name="xps")
                    for dt in range(NDT):
                        pe.matmul(xps[:], xpTs[d][dt][:],
                                  u_cat[dt][:, d * L + n * 512: d * L + (n + 1) * 512],
                                  start=(dt == 0), stop=(dt == 1))
                    act.activation(xsb[:, n * 512:(n + 1) * 512], xps[:], ActFn.Copy)
                sync.dma_start(xdbl_loc[d], xsb[:])
            pool.collective_compute(
                "AllReduce", AluOp.add,
                replica_groups=[[0, 1, 2, 3], [4, 5, 6, 7]],
                ins=[xdbl_loc[:].opt()],
                outs=[xdbl_red[:].opt()],
            )

        def phase_B2(rep, st):
            """dt_proj, selective scan, gate, out_proj, ReduceScatter."""
            rp2 = rep % 2
            xdbl_red = scr[rp2]["xdbl_red"]
            out_loc, out_rs = scr[rp2]["out_loc"], scr[rp2]["out_rs"]
            u_cat = st["u_cat"]
            if upto in ('dw', 'xdbl'):
                if upto == 'xdbl':
                    dbg_out(u_cat[0][:, 0:L])
                return
            # ---------------- dt_proj -> m = -softplus = ln(sigmoid(-x)) --------
            m_cat = [mid.tile([P, T2], BF, tag=f"m{dt}", name=f"m{dt}") for dt in range(NDT)]
            for d in range(2):
                dtfb = scanp.tile([32, L], BF, tag="dtfb", bufs=1)
                sync.dma_start(dtfb[:], xdbl_red[d, 0:32, :])
                for dt in range(NDT):
                    for n in range(2):
                        ps = psum.tile([P, 512], F32, tag="ps_main", name="ps")
                        pe.matmul(ps[:], dtTs[d][:, dt * 128:(dt + 1) * 128],
                                  dtfb[:, n * 512:(n + 1) * 512])
                        sgm = scanp.tile([P, 512], F32, tag="sgm", bufs=1)
                        act.activation(sgm[:], ps[:], ActFn.Sigmoid, scale=-1.0,
                                       bias=ndtbs[d][dt][:])
                        act.activation(m_cat[dt][:, d * L + n * 512: d * L + (n + 1) * 512],
                                       sgm[:], ActFn.Ln)

            # ux = -(m * u) = delta * u; then poison m[,0]/m[,L] so exp -> 0
            ux = [mid.tile([P, T2], BF, tag=f"ux{dt}", name=f"ux{dt}") for dt in range(NDT)]
            for dt in range(NDT):
                vec.scalar_tensor_tensor(ux[dt][:], m_cat[dt][:], -1.0, u_cat[dt][:],
                                         AluOp.mult, AluOp.mult)
            for dt in range(NDT):
                vec.memset(m_cat[dt][:, 0:1], -30.0)
                vec.memset(m_cat[dt][:, L:L + 1], -30.0)

            if upto == 'dt':
                dbg_out(m_cat[0][:, 0:L])
                return
            yield
            # ---------------- selective scan ----------------
            # Direction-outer: B_s/C_s broadcast once per (d, s) and shared by
            # both d-tiles; each (d, dt) accumulates y into a 2-bank PSUM half.
            # B_s rows live at xdbl_red[d, 32+s, :], C_s at xdbl_red[d, 48+s, :]
            xr_ap = xdbl_red[:]
            yc = {}
            ygate = []
            for d in range(2):
                ps_d = [psum.tile([P, L], F32, tag="ps_yd", name=f"ps_{d}{dt}",
                                  bufs=2) for dt in range(NDT)]
                for sp in range(8):  # s-pair (2*sp, 2*sp+1)
                    if sp in (2, 4, 6):
                        yield
                    s0 = 2 * sp
                    # one DMA per (d, s-pair): [B_s|B_s+1|C_s|C_s+1]; row pairs
                    # are contiguous in xdbl_red so descriptors stay simple
                    if PROBE == 'nobcast' and sp > 0:
                        bc = bc_prev
                    else:
                        bc = scanp.tile([P, 4 * L], BF, tag="Bs", bufs=2)
                        sync.dma_start(
                            bc[:],
                            _ap_bcast_dram(xr_ap.tensor,
                                           xr_ap.offset + (d * 64 + 32 + s0) * L,
                                           [[0, P], [16 * L, 2], [1, 2 * L]]),
                        )
                        bc_prev = bc
                    for dt in range(NDT):
                        sl = slice(d * L, (d + 1) * L)
                        a_s = scanp.tile([P, 2 * L], BF, tag="a_s")
                        act.activation(a_s[:, 0:L], m_cat[dt][:, sl], ActFn.Exp,
                                       scale=float(-a_vals[s0]))
                        act.activation(a_s[:, L:2 * L], m_cat[dt][:, sl], ActFn.Exp,
                                       scale=float(-a_vals[s0 + 1]))
                        # ux repeated across the pair via a stride-0 middle dim
                        uxh = ux[dt][:, sl]
                        ux2 = bass.AP(tensor=uxh.tensor, offset=uxh.offset,
                                      ap=[list(uxh.ap[0]), [0, 2], [1, L]])
                        b_s = scanp.tile([P, 2 * L], BF, tag="b_s")
                        bp = POOL_MULTS and (dt == 0)
                        gp = POOL_MULTS and (dt == 1 and sp % 2 == 1)
                        (pool if bp else vec).tensor_mul(b_s[:], ux2, bc[:, 0:2 * L])
                        h_s = scanp.tile([P, 2 * L], BF, tag="h_s")
                        if PROBE == 'noscan':
                            vec.tensor_mul(h_s[:], a_s[:], b_s[:])
                        else:
                            vec.tensor_tensor_scan(h_s[:], a_s[:], b_s[:], 0.0,
                                                   AluOp.mult, AluOp.add)
                        gs = scanp.tile([P, 2 * L], BF, tag="b_s", name="gs")
                        (pool if gp else vec).tensor_mul(gs[:], h_s[:],
                                                         bc[:, 2 * L:4 * L])
                        for j in range(4):
                            pe.matmul(ps_d[dt][:, (j % 2) * 512:(j % 2) * 512 + 512],
                                      ident[:], gs[:, j * 512:(j + 1) * 512],
                                      start=(sp == 0 and j < 2), stop=False)
                # D-term (diag D matmul on u) closes each accumulation
                for dt in range(NDT):
                    for ch in range(2):
                        pe.matmul(ps_d[dt][:, ch * 512:(ch + 1) * 512], dDs[d][dt][:],
                                  u_cat[dt][:, d * L + ch * 512: d * L + (ch + 1) * 512],
                                  start=False, stop=True)
                    t = onep.tile([P, L], BF, tag=f"yc{d}{dt}", name=f"yc{d}{dt}")
                    act.activation(t[:], ps_d[dt][:], ActFn.Copy)
                    yc[(d, dt)] = t
                if d == 0:
                    yield
            for dt in range(NDT):
                ysum = onep.tile([P, L], BF, tag="ysum")
                vec.tensor_add(ysum[:], yc[(0, dt)][:],
                               yc[(1, dt)][:, L - 1::-1])
                yg = scanp.tile([P, L], BF, tag="yg")
                vec.tensor_mul(yg[:], ysum[:], st["zs"][dt][:])
                ygate.append(yg)

            if upto == 'scan':
                dbg_out(yc[(0, 0)])
                return

            # ---------------- out_proj + res/4 -> bf16 ReduceScatter ------------
            rd = res_dram[st["rp3"]]
            for m in range(4):
                osb = onep.tile([P, L], BF, tag="osb", bufs=2)
                resc = scanp.tile([P, L], BF, tag="resc", bufs=1)
                sync.dma_start(resc[:], rd[m * 128:(m + 1) * 128, :])
                for n in range(2):
                    ps = psum.tile([P, 512], F32, tag="ps_main", name="ps")
                    for dt in range(NDT):
                        pe.matmul(ps[:], outTs[dt][:, m * 128:(m + 1) * 128],
                                  ygate[dt][:, n * 512:(n + 1) * 512],
                                  start=(dt == 0), stop=False)
                    pe.matmul(ps[:], idq[:], resc[:, n * 512:(n + 1) * 512],
                              start=False, stop=True)
                    act.activation(osb[:, n * 512:(n + 1) * 512], ps[:], ActFn.Copy)
                sync.dma_start(out_loc[m * 128:(m + 1) * 128, :], osb[:])
            pool.collective_compute(
                "ReduceScatter", AluOp.add,
                replica_groups=[[0, 1, 2, 3], [4, 5, 6, 7]],
                ins=[out_loc[:].opt()],
                outs=[out_rs[:].opt()],
            )
            sync.dma_start(out_ext[:], out_rs[:])

        # Skewed emission: A(k) | B1(k-1) | B2(k-2).  In-order engines then
        # interleave rep k's stem (PE/Act) with rep k-2's scan (DVE/Pool),
        # and rep k-1's AllReduce overlaps both.
        def step(g):
            if g is None:
                return False
            try:
                next(g)
                return True
            except StopIteration:
                return False

        states = {}
        if upto in ('stem', 'inproj'):
            for k in range(reps):
                states[k] = {}
                for _ in phase_A(k, states[k]):
                    pass
        else:
            for k in range(reps + 2):
                gb = phase_B2(k - 2, states.pop(k - 2)) if 2 <= k <= reps + 1 else None
                ga = None
                if k < reps:
                    states[k] = {}
                    ga = phase_A(k, states[k])
                step(gb); step(ga); step(gb)
                if 1 <= k <= reps:
                    phase_B1(k - 1, states[k - 1])
                alive = True
                while alive:
                    alive = step(gb)
                    alive = step(ga) or alive

    if split_waits:
        split_excess_waits(nc)
    return nc


def prep_inputs(inputs):
    """Host-side sharding/weight prep.  Returns (a_vals, in_maps)."""
    f32 = lambda a: np.ascontiguousarray(np.asarray(a, np.float32))
    bf = lambda a: np.ascontiguousarray(np.asarray(a, np.float32).astype(BF16))

    A_f = -np.exp(f32(inputs["Alog_f"]))
    A_r = -np.exp(f32(inputs["Alog_r"]))
    assert np.abs(A_f - A_f[0:1]).max() < 1e-5, "A not d-independent"
    assert np.abs(A_f - A_r).max() < 1e-5, "A_f != A_r"
    a_vals = [float(v) for v in A_f[0]]

    x = f32(inputs["x"])
    w1 = f32(inputs["conv1_w"]); w2 = f32(inputs["conv2_w"]); w3 = f32(inputs["conv3_w"])
    w1T = bf(np.transpose(w1, (2, 1, 0)).reshape(3, 6, 128, 128))
    w2T = bf(np.transpose(w2, (2, 1, 0)).reshape(3, 1, 128, 256))
    w3T = bf(np.transpose(w3, (2, 1, 0)).reshape(3, 2, 128, 512))
    onehot = np.zeros((3, 128, 32), np.float32)
    for i, cg in enumerate((4, 8, 16)):
        onehot[i, np.arange(128), np.arange(128) // cg] = 1.0
    ln_g = f32(inputs["ln_g"]); ln_b = f32(inputs["ln_b"])
    ipw = f32(inputs["in_proj_w"])
    opw = f32(inputs["out_proj_w"])

    common = dict(
        w1T=w1T, w2T=w2T, w3T=w3T,
        cb1=f32(inputs["conv1_b"]).reshape(128, 1),
        cb2=f32(inputs["conv2_b"]).reshape(256, 1),
        cb3=f32(inputs["conv3_b"]).reshape(512, 1),
        gng1=f32(inputs["gn1_g"]).reshape(128, 1),
        gnb1=f32(inputs["gn1_b"]).reshape(128, 1),
        gng2=f32(inputs["gn2_g"]).reshape(256, 1),
        gnb2=f32(inputs["gn2_b"]).reshape(256, 1),
        gng3=f32(inputs["gn3_g"]).reshape(512, 1),
        gnb3=f32(inputs["gn3_b"]).reshape(512, 1),
        onehot=bf(onehot),
        onehotE=bf(np.transpose(onehot, (0, 2, 1))),
        ones_col=bf(np.ones((128, 1), np.float32)),
        identw=bf(np.eye(128, dtype=np.float32)),
        idq_w=bf(np.eye(128, dtype=np.float32) / NGRP),
    )

    in_maps = []
    for core in range(NCORES):
        b, grp = core // NGRP, core % NGRP
        rows = np.arange(grp * DSH, (grp + 1) * DSH)
        sel = np.concatenate([rows, DI + rows])
        Wsel = ipw[sel] * ln_g[None, :]
        inprojT = bf(Wsel.T.reshape(4, 128, 2 * DSH))
        augTm = bf(np.stack([Wsel.sum(1), ipw[sel] @ ln_b]))
        xpTm = np.stack([
            bf(f32(inputs[f"xp_w_{s}"])[:, rows].T.reshape(2, 128, 64))
            for s in ("f", "r")])
        dtTm = np.stack([
            bf(f32(inputs[f"dt_w_{s}"])[rows].T) for s in ("f", "r")])
        ndtbm = np.stack([
            -f32(inputs[f"dt_b_{s}"])[rows].reshape(DSH, 1) for s in ("f", "r")])
        cvwDm = np.zeros((2, NDT, DC, 128, 128), np.float32)
        for di, s in enumerate(("f", "r")):
            wv = f32(inputs[f"cv_w_{s}"])[rows, 0]          # (DSH, 4)
            for dt in range(NDT):
                for k in range(DC):
                    np.fill_diagonal(cvwDm[di, dt, k], wv[dt * 128:(dt + 1) * 128, k])
        cvbm = np.stack([
            f32(inputs[f"cv_b_{s}"])[rows].reshape(DSH, 1) for s in ("f", "r")])
        dDm = np.zeros((2, NDT, 128, 128), np.float32)
        for di, s in enumerate(("f", "r")):
            Dv = f32(inputs[f"D_{s}"])[rows]
            for dt in range(NDT):
                np.fill_diagonal(dDm[di, dt], Dv[dt * 128:(dt + 1) * 128])
        outTm = bf(opw[:, rows].T.reshape(2, 128, DM))
        xpadded = bf(np.pad(x[b], ((0, 0), (1, 1))))
        m = dict(common)
        m.update(x=xpadded, inprojT=inprojT, augT=augTm, xpT=xpTm, dtT=dtTm,
                 ndtb=ndtbm, cvwD=bf(cvwDm), cvb=cvbm, dDiag=bf(dDm), outT=outTm)
        in_maps.append(m)
    return a_vals, in_maps


def assemble_output(results):
    """results: list of 8 dicts with 'out' (128, L).  Each group of 4 cores
    holds the 4 row-quarters of its batch sample."""
    full = []
    for b in range(B):
        full.append(np.concatenate(
            [np.asarray(results[b * NGRP + g]["out"], np.float32)
             for g in range(NGRP)], axis=0))
    return np.ascontiguousarray(np.stack(full).astype(np.float32))


def kernel(**inputs) -> np.ndarray:
    from concourse.bass_utils import run_bass_kernel_spmd
    a_vals, in_maps = prep_inputs(inputs)
    nc = build_program(a_vals)
    res = run_bass_kernel_spmd(nc, in_maps, list(range(NCORES)))
    return assemble_output(res.results)


if __name__ == "__main__":
    import reference as R
    import jax
    with jax.default_device(jax.devices("cpu")[0]):
        inp = {k: np.asarray(v) for k, v in R.setup_inputs().items()}
        ref = np.asarray(R.reference(**R.setup_inputs()))
    got = kernel(**inp)
    err = np.abs(got - ref).max() / np.abs(ref).max()
    print("Relative error:", err)
